# revision 21
# baseline (speedup 1.0000x reference)
"""Trainium2 Bass kernel for nn_CrossAttentionClassifier.

Strategy
--------
The reference network with q_len = kv_len = 1 attention degenerates into a
pure MLP:

    z_m = mut @ Wm' + bm'          (centered: LN mean-subtraction folded into W)
    z_c = ctx @ Wc' + bc'
    h_m = relu(z_m * rsqrt(mean(z_m^2)+eps)) ; h_c likewise
    pre1 = h_c @ CA + h_m @ CM + d (attention V/out projections + c1 folded)
    z1   = pre1 * rsqrt(mean(pre1^2)+eps)
    h1   = relu(z1)                (c1_g==1, c1_be==0; g folded into c2_w)
    h2   = relu(h1 @ c2_w + c2_b)
    out  = h2 @ c3_w + c3_b

All weight folding (products of the tiny 256x256 projection chains and the
centering projector I - 11^T/256) happens on host in float64; the batch-heavy
work runs on 8 NeuronCores, data-parallel over the 65536-row batch.

Device layout: batch on SBUF partitions, features on the free axis, so both
layernorms reduce along the free dimension.  Activations are transposed
128x128 via the PE between layers so the next matmul's contraction dim lands
on partitions.  All matmuls run in bf16 (fp32 PSUM accumulate).

v3 scheduling (trace-driven; the PE matmul stream is the bottleneck at
~107 ns per N=256 MM, i.e. the 1 col/cycle floor):
 - x is pre-blocked on host into one [128, (st, embed, k, col)] image so each
   (supertile, embed) loads with a single 128-line DMA: descriptor expansion
   on the Sync engine drops from 3-11 us to ~0.6 us per load, and lines are
   4-17 KB/partition (full line rate);
 - everything rides the Sync/HWDGE queue -- no GPSIMD SWDGE (its library
   preamble delayed kernel start ~5 us and its drain added ~6 us at the end);
 - supertile widths 128/128/256 at the start (fast first matmul, early HAM
   warm-up) and 256/128/128 at the end (short drain chain);
 - the first supertile's x and the embed weights arrive as interleaved
   k-pieces so the first matmul issues ~4 us in;
 - outputs accumulate in one [1, 8192] SBUF tile, stored with a single DMA
   at the end;
 - LN stats: one ACT Square+accumulate per z (PSUM), bn_stats for s1;
   copybacks/relu evicts merged into fewer, larger ACT/DVE ops.
"""

import numpy as np
import ml_dtypes

import concourse.bass as bass
import concourse.mybir as mybir
import concourse.tile as tile
from concourse.bass_utils import run_bass_kernel_spmd

BF16 = ml_dtypes.bfloat16
F32 = np.float32

N_CORES = 8
B = 65536
IN_DIM = 2056
E = 256
EPS = 1e-5
KP = 2176          # feature dim padded to 17*128 (incl. bias row at 2056)
KCH = KP // 128    # 17
ROWS = B // N_CORES   # 8192 rows per core

SUP = [128, 128, 256] + [512] * 14 + [256, 128, 128]   # supertile widths
assert sum(SUP) == ROWS
X2COLS = 2 * KCH * ROWS
K_PIECES0 = [(0, 3), (3, 8), (8, 13), (13, KCH)]   # st0 weight/x interleave

_BF = mybir.dt.bfloat16
_F32 = mybir.dt.float32
_AF = mybir.ActivationFunctionType

# ---- packed bf16 const image column offsets ----
_WM0 = 0
_WC0 = _WM0 + KCH * E          # 4352
_WMID0 = _WC0 + KCH * E        # 8704
_C2W0 = _WMID0 + 4 * E         # 9728
_C3W0 = _C2W0 + 2 * 64         # 9856
_ID0 = _C3W0 + 1               # 9857
_WCOLS = _ID0 + 128            # 9985

# ---- packed f32 const image columns: d broadcast | c2_b | c3_b | eps ----
_FCOLS = E + 3


def _x2_offsets():
    """dram column offset of each (st, embed) segment in the x2 image."""
    offs = {}
    col = 0
    for st, w in enumerate(SUP):
        offs[(st, 0)] = col
        offs[(st, 1)] = col + KCH * w
        col += 2 * KCH * w
    assert col == X2COLS
    return offs


_X2OFF = _x2_offsets()


def _build_nc():
    nc = bass.Bass()

    x2 = nc.dram_tensor("x2", [128, X2COLS], _BF, kind="ExternalInput")
    wpack = nc.dram_tensor("wpack", [128, _WCOLS], _BF, kind="ExternalInput")
    fpack = nc.dram_tensor("fpack", [128, _FCOLS], _F32, kind="ExternalInput")
    out = nc.dram_tensor("out", [1, ROWS], _F32, kind="ExternalOutput")

    from contextlib import ExitStack

    with tile.TileContext(nc) as tc, ExitStack() as ctx:
        consts = ctx.enter_context(tc.tile_pool(name="consts", bufs=1))
        xpool = ctx.enter_context(tc.tile_pool(name="xpool", bufs=3))
        zpool = ctx.enter_context(tc.tile_pool(name="zpool", bufs=3))
        sqpool = ctx.enter_context(tc.tile_pool(name="sqpool", bufs=3))
        hpool = ctx.enter_context(tc.tile_pool(name="hpool", bufs=4))
        spool = ctx.enter_context(tc.tile_pool(name="spool", bufs=6))
        pe_mc = ctx.enter_context(tc.tile_pool(name="pe_mc", bufs=2, space="PSUM"))
        pe_p1 = ctx.enter_context(tc.tile_pool(name="pe_p1", bufs=2, space="PSUM"))
        pe_t = ctx.enter_context(tc.tile_pool(name="pe_t", bufs=2, space="PSUM"))
        pe_small = ctx.enter_context(tc.tile_pool(name="pe_small", bufs=2, space="PSUM"))

        wsb = consts.tile([128, _WCOLS], _BF, tag="wsb")
        fsb = consts.tile([128, _FCOLS], _F32, tag="fsb")
        out_acc = consts.tile([1, ROWS], _F32, tag="out_acc")

        # ---- HAM warm-up: dummy matmuls over a memset scratch keep the PE
        # clock-gate at 8/8 through the ~8 us DMA preroll, so the first real
        # matmuls run at 2.4 GHz instead of 1.2 GHz.
        scr = consts.tile([128, 128], _BF, tag="warm_scr")
        nc.vector.memset(scr, 0.0)
        pwarm = pe_t.tile([128, 2, 128], _F32, tag="pt")
        for _ in range(110):
            nc.tensor.matmul(pwarm[:, 0, :], lhsT=scr, rhs=scr,
                             start=True, stop=True)

        def load_w(c0, c1):
            # weights ride the Scalar-engine HWDGE ring, parallel to the
            # x ring on Sync, so they never delay an x descriptor
            nc.scalar.dma_start(out=wsb[:, c0:c1], in_=wpack[:, c0:c1])

        def load_x_piece(xt, st, e, k0, k1, w):
            src = x2[:, bass.ds(_X2OFF[(st, e)] + k0 * w, (k1 - k0) * w)]
            nc.sync.dma_start(
                out=xt[:, k0:k1, 0:w],
                in_=src.rearrange("p (k c) -> p k c", c=w))

        # views into the packed images
        wm_sb = wsb[:, _WM0:_WC0].rearrange("p (k j) -> p k j", j=E)
        wc_sb = wsb[:, _WC0:_WMID0].rearrange("p (k j) -> p k j", j=E)
        wmid_sb = wsb[:, _WMID0:_C2W0].rearrange("p (k j) -> p k j", j=E)
        c2w_sb = wsb[:, _C2W0:_C3W0].rearrange("p (k j) -> p k j", j=64)
        c3w_sb = wsb[:64, _C3W0:_C3W0 + 1]
        ident = wsb[:, _ID0:_ID0 + 128]
        d_bc = fsb[:, 0:E]
        c2b_ap = fsb[:64, E:E + 1]
        c3b_ap = fsb[:1, E + 1:E + 2]
        eps_ap = fsb[:, E + 2:E + 3]

        def sumsq_psum(ph):
            """sum(x^2) along free axis of a [128, E] PSUM tile (one ACT op;
            DVE cannot dual-read PSUM)."""
            sq = sqpool.tile([128, E], _BF, tag="sq")
            ssq = spool.tile([128, 1], _F32, tag="ssq")
            nc.scalar.activation(out=sq, in_=ph, func=_AF.Square,
                                 accum_out=ssq)
            return ssq

        def rsqrt_mean(ssq, scale):
            """rsqrt(ssq*scale + eps): ACT sqrt + DVE reciprocal."""
            sd = spool.tile([128, 1], _F32, tag="sd")
            nc.scalar.activation(out=sd, in_=ssq, func=_AF.Sqrt,
                                 bias=eps_ap, scale=scale)
            rs = spool.tile([128, 1], _F32, tag="rs")
            nc.vector.reciprocal(out=rs, in_=sd)
            return rs

        h2cats = {}      # st -> [64, 512] bf16 accumulator
        h1tiles = {}     # pair key -> [128, 2, npair, 128] tile

        def stage_a(blk):
            """Embed matmuls for one 128-col block (PE only)."""
            x_m, x_c, bcol = blk["xap"]
            pmc = pe_mc.tile([128, 2, E], _F32, tag="mc")
            for i, (x_sb, w_sb) in enumerate(((x_m, wm_sb), (x_c, wc_sb))):
                for k in range(KCH):
                    nc.tensor.matmul(
                        pmc[:, i, :], lhsT=x_sb[:, k, bcol], rhs=w_sb[:, k, :],
                        start=(k == 0), stop=(k == KCH - 1))
            blk["pmc"] = pmc

        def stage_a2(blk):
            """LN chain + z evict (DVE/ACT only)."""
            pmc = blk.pop("pmc")
            ssq = [sumsq_psum(pmc[:, i, :]) for i in range(2)]
            rs = [rsqrt_mean(s, 1.0 / E) for s in ssq]
            zs = []
            for i in range(2):
                z = zpool.tile([128, E], _BF, tag=f"z{i}")
                nc.scalar.activation(out=z, in_=pmc[:, i, :],
                                     func=_AF.Relu, scale=rs[i])
                zs.append(z)
            blk["z"] = zs

        def stage_b(blk):
            """z transposes on PE + merged copybacks -> hT."""
            pt = pe_t.tile([128, 4, 128], _BF, tag="pt")
            zm, zc = blk.pop("z")
            for chv in range(2):
                nc.tensor.transpose(pt[:, chv, :], zm[:, bass.ts(chv, 128)], ident)
            for chv in range(2):
                nc.tensor.transpose(pt[:, 2 + chv, :], zc[:, bass.ts(chv, 128)], ident)
            ht_m = hpool.tile([128, 2, 128], _BF, tag="hT0")
            ht_c = hpool.tile([128, 2, 128], _BF, tag="hT1")
            nc.scalar.copy(out=ht_m, in_=pt[:, 0:2, :])
            nc.vector.tensor_copy(out=ht_c, in_=pt[:, 2:4, :])
            blk["ht"] = (ht_m, ht_c)

        def stage_c(blk):
            """Mid matmuls + d add + z1 chain."""
            ht_m, ht_c = blk.pop("ht")
            p1 = pe_p1.tile([128, E], _F32, tag="p1")
            nc.tensor.matmul(p1, lhsT=ht_c[:, 0, :], rhs=wmid_sb[:, 0, :], start=True, stop=False)
            nc.tensor.matmul(p1, lhsT=ht_c[:, 1, :], rhs=wmid_sb[:, 1, :], start=False, stop=False)
            nc.tensor.matmul(p1, lhsT=ht_m[:, 0, :], rhs=wmid_sb[:, 2, :], start=False, stop=False)
            nc.tensor.matmul(p1, lhsT=ht_m[:, 1, :], rhs=wmid_sb[:, 3, :], start=False, stop=True)
            s1 = spool.tile([128, E], _F32, tag="s1")
            nc.vector.tensor_add(out=s1, in0=p1, in1=d_bc)
            stats = spool.tile([128, 6], _F32, tag="stats")
            nc.vector.bn_stats(out=stats, in_=s1)
            mv = spool.tile([128, 2], _F32, tag="mv")
            nc.vector.bn_aggr(out=mv, in_=stats)
            rs1 = rsqrt_mean(mv[:, 1:2], 1.0)
            z1 = zpool.tile([128, E], _BF, tag="zmid")
            nc.vector.tensor_scalar_mul(out=z1, in0=s1, scalar1=rs1)
            blk["z1"] = z1

        def stage_d(blk):
            """z1 transposes on PE + h1 = relu(z1T) merged copyback.
            (c1 affine is identity; c1_g is folded into c2_w on host.)"""
            z1 = blk.pop("z1")
            pt = pe_t.tile([128, 4, 128], _BF, tag="pt")
            key = blk["pair"]
            if blk["par"] == 0:
                h1tiles[key] = hpool.tile(
                    [128, 2, blk["npair"], 128], _BF, tag="h1T", name="h1pair")
            h1 = h1tiles[key]
            for chv in range(2):
                nc.tensor.transpose(pt[:, chv, :], z1[:, bass.ts(chv, 128)], ident)
            nc.vector.tensor_scalar_max(
                out=h1[:, :, blk["par"], :], in0=pt[:, 0:2, :], scalar1=0.0)

        def stage_e(blk):
            """c2 matmul per block pair + h2 relu evict."""
            npair = blk["npair"]
            if blk["par"] != npair - 1:
                return
            st = blk["st"]
            h1 = h1tiles.pop(blk["pair"])
            n = npair * 128
            ph2 = pe_small.tile([64, n], _F32, tag="small")
            nc.tensor.matmul(ph2, lhsT=c2w_sb[:, 0, :], rhs=h1[:, 0, :, :], start=True, stop=False)
            nc.tensor.matmul(ph2, lhsT=c2w_sb[:, 1, :], rhs=h1[:, 1, :, :], start=False, stop=True)
            if st not in h2cats:
                h2cats[st] = hpool.tile([64, 512], _BF, tag="h2T", name="h2cat")
            nc.scalar.activation(
                out=h2cats[st][:, bass.ds(blk["pcol"], n)], in_=ph2,
                func=_AF.Relu, bias=c2b_ap)

        def stage_f(blk):
            """Batched c3 matmul over a whole supertile + out_acc write."""
            if not blk["last_in_st"]:
                return
            st, w = blk["st"], blk["stw"]
            po = pe_small.tile([1, w], _F32, tag="small")
            nc.tensor.matmul(po, lhsT=c3w_sb, rhs=h2cats.pop(st)[:, 0:w],
                             start=True, stop=True)
            nc.vector.tensor_scalar_add(
                out=out_acc[:, bass.ds(blk["st0col"], w)], in0=po,
                scalar1=c3b_ap)

        stages = [stage_a2, stage_b, stage_c, stage_d, stage_e, stage_f]
        pipe = []
        col = 0
        for st, w in enumerate(SUP):
            nb = w // 128
            x_m = xpool.tile([128, KCH, 512], _BF, tag="x_m")
            x_c = xpool.tile([128, KCH, 512], _BF, tag="x_c")
            if st == 0:
                # weights stream on the Scalar ring (k-pieces so the first
                # matmul only waits for wm chunks 0-2); x pieces on Sync
                for k0, k1 in K_PIECES0:
                    load_w(_WM0 + k0 * E, _WM0 + k1 * E)
                    load_w(_WC0 + k0 * E, _WC0 + k1 * E)
                load_w(_WMID0, _WCOLS)
                nc.scalar.dma_start(out=fsb, in_=fpack[:])
                for k0, k1 in ((0, 3), (3, KCH)):
                    load_x_piece(x_m, st, 0, k0, k1, w)
                    load_x_piece(x_c, st, 1, k0, k1, w)
            else:
                load_x_piece(x_m, st, 0, 0, KCH, w)
                load_x_piece(x_c, st, 1, 0, KCH, w)

            for bb in range(nb):
                blk = {
                    "st": st, "stw": w, "st0col": col,
                    "pair": (st, bb // 2), "par": bb % 2,
                    "npair": min(2, nb - (bb // 2) * 2),
                    "pcol": (bb // 2) * 256,
                    "last_in_st": bb == nb - 1,
                    "xap": (x_m, x_c, bass.ts(bb, 128)),
                }
                stage_a(blk)
                pipe.append(blk)
                for depth, fn in enumerate(stages, start=2):
                    if len(pipe) >= depth:
                        fn(pipe[-depth])
                if len(pipe) > len(stages) + 1:
                    pipe.pop(0)
            col += w
        # drain: stage j (depth j+2) still owes the last j+1 blocks
        for j, fn in enumerate(stages):
            for blk in pipe[-(j + 1):]:
                fn(blk)
        nc.scalar.dma_start(out=out[:, :], in_=out_acc)

    return nc


def _legalize_waits(nc):
    """Split multi-semaphore waits: this walrus build accepts at most one
    sync-wait per instruction (two on EventSemaphore), so excess waits are
    hoisted into preceding EventSemaphore instructions on the same engine."""
    for bb in nc.main_func.blocks:
        new_insts = []
        changed = False
        for inst in bb.instructions:
            si = inst.sync_info
            if si is not None and si.on_wait:
                cap = 2 if isinstance(inst, mybir.InstEventSemaphore) else 1
                waits = list(si.on_wait)
                while len(waits) > cap:
                    spill, waits = waits[:2], waits[2:]
                    ev = mybir.InstEventSemaphore(
                        name=nc.get_next_instruction_name(),
                        ins=[], outs=[],
                        engine=inst.engine,
                        sync_info=mybir.SyncInfo(on_wait=spill, on_update=[]),
                    )
                    new_insts.append(ev)
                    changed = True
                si.on_wait = waits
            new_insts.append(inst)
        if changed:
            bb.instructions[:] = new_insts


_NC_CACHE = {}


def _get_nc():
    if "nc" not in _NC_CACHE:
        nc = _build_nc()
        _legalize_waits(nc)
        _NC_CACHE["nc"] = nc
    return _NC_CACHE["nc"]


def _fold_weights(inp):
    f8 = lambda k: np.asarray(inp[k]).astype(np.float64)
    P_c = np.eye(E) - 1.0 / E

    me_w, me_b = f8("me_w"), f8("me_b")
    ce_w, ce_b = f8("ce_w"), f8("ce_b")
    Wm = np.zeros((KP, E))
    Wm[:IN_DIM] = me_w @ P_c
    Wm[IN_DIM] = me_b @ P_c
    Wc = np.zeros((KP, E))
    Wc[:IN_DIM] = ce_w @ P_c
    Wc[IN_DIM] = ce_b @ P_c

    c1_w, c1_b = f8("c1_w"), f8("c1_b")
    A0 = f8("ca_in_w")[:, 2 * E:] @ f8("ca_out_w")
    a0 = f8("ca_in_b")[2 * E:] @ f8("ca_out_w") + f8("ca_out_b")
    S0 = f8("sa_in_w")[:, 2 * E:] @ f8("sa_out_w")
    s0 = f8("sa_in_b")[2 * E:] @ f8("sa_out_w") + f8("sa_out_b")
    CA = (A0 @ c1_w[:E]) @ P_c
    CM = (S0 @ c1_w[E:]) @ P_c
    d = (a0 @ c1_w[:E] + s0 @ c1_w[E:] + c1_b) @ P_c

    # fold c1's affine (g, be) into the c2 projection: with be == 0 and
    # g > 0, relu(g*z + be) @ c2_w == relu(z) @ (g[:,None] * c2_w)
    c1_g, c1_be = f8("c1_g"), f8("c1_be")
    assert np.all(c1_be == 0.0) and np.all(c1_g > 0.0)
    c2_w = c1_g[:, None] * f8("c2_w")

    # ---- bf16 packed image ----
    w = np.zeros((128, _WCOLS), BF16)

    def chunked(mat, ncols):       # [k*128, ncols] -> [128, k*ncols]
        k = mat.shape[0] // 128
        return mat.reshape(k, 128, ncols).transpose(1, 0, 2).reshape(128, k * ncols)

    w[:, _WM0:_WC0] = chunked(Wm, E).astype(BF16)
    w[:, _WC0:_WMID0] = chunked(Wc, E).astype(BF16)
    w[:, _WMID0:_C2W0] = chunked(np.vstack([CA, CM]), E).astype(BF16)
    w[:, _C2W0:_C3W0] = chunked(c2_w, 64).astype(BF16)
    w[:64, _C3W0:_C3W0 + 1] = f8("c3_w").astype(BF16)
    w[:, _ID0:_ID0 + 128] = np.eye(128, dtype=BF16)

    # ---- f32 packed image: d broadcast | c2_b | c3_b | eps ----
    f = np.zeros((128, _FCOLS), F32)
    f[:, 0:E] = d.astype(F32)[None, :]
    f[:64, E] = np.asarray(inp["c2_b"]).astype(F32)
    f[0, E + 1] = float(np.asarray(inp["c3_b"]).reshape(-1)[0])
    f[:, E + 2] = EPS
    return {"wpack": w, "fpack": f}


def _shard_x(x):
    """x [B, 2, IN_DIM] f32 -> per-core x2 image [128, X2COLS] bf16.

    Layout: for each supertile st (width w) and embed e, a contiguous
    [128, KCH*w] segment holding that supertile's feature-major x, k-major
    within the segment -- so every (st, e) DMA is one 4-17 KB line per
    partition, and k-range sub-slices stay contiguous."""
    maps = []
    for i in range(N_CORES):
        sl = x[i * ROWS:(i + 1) * ROWS]          # [ROWS, 2, IN_DIM]
        xe = []
        for e in range(2):
            t = np.zeros((KP, ROWS), BF16)
            t[:IN_DIM] = np.ascontiguousarray(sl[:, e, :]).astype(BF16).T
            t[IN_DIM] = 1
            xe.append(t)
        segs = []
        col = 0
        for st, w in enumerate(SUP):
            for e in range(2):
                # [KCH, 128, w] -> [128, KCH, w] -> [128, KCH*w]
                seg = xe[e][:, col:col + w].reshape(KCH, 128, w)
                segs.append(seg.transpose(1, 0, 2).reshape(128, KCH * w))
            col += w
        x2 = np.concatenate(segs, axis=1)
        assert x2.shape == (128, X2COLS)
        maps.append(np.ascontiguousarray(x2))
    return maps


def kernel(**inputs):
    x = np.asarray(inputs["x"], dtype=np.float32)
    weights = _fold_weights(inputs)
    shards = _shard_x(x)
    in_maps = [{"x2": x2, **weights} for x2 in shards]

    nc = _get_nc()
    res = run_bass_kernel_spmd(nc, in_maps, list(range(N_CORES)))
    outs = [np.asarray(r["out"]).reshape(ROWS) for r in res.results]
    return np.concatenate(outs).reshape(B, 1).astype(np.float32)


# revision 25
# speedup vs baseline: 1.0039x; 1.0039x over previous
"""Trainium2 Bass kernel for nn_CrossAttentionClassifier.

Strategy
--------
The reference network with q_len = kv_len = 1 attention degenerates into a
pure MLP:

    z_m = mut @ Wm' + bm'          (centered: LN mean-subtraction folded into W)
    z_c = ctx @ Wc' + bc'
    h_m = relu(z_m * rsqrt(mean(z_m^2)+eps)) ; h_c likewise
    pre1 = h_c @ CA + h_m @ CM + d (attention V/out projections + c1 folded)
    z1   = pre1 * rsqrt(mean(pre1^2)+eps)
    h1   = relu(z1)                (c1_g==1, c1_be==0; g folded into c2_w)
    h2   = relu(h1 @ c2_w + c2_b)
    out  = h2 @ c3_w + c3_b

All weight folding (products of the tiny 256x256 projection chains and the
centering projector I - 11^T/256) happens on host in float64; the batch-heavy
work runs on 8 NeuronCores, data-parallel over the 65536-row batch.

Device layout: batch on SBUF partitions, features on the free axis, so both
layernorms reduce along the free dimension.  Activations are transposed
128x128 via the PE between layers so the next matmul's contraction dim lands
on partitions.  All matmuls run in bf16 (fp32 PSUM accumulate).

v3 scheduling (trace-driven; the PE matmul stream is the bottleneck at
~107 ns per N=256 MM, i.e. the 1 col/cycle floor):
 - x is pre-blocked on host into one [128, (st, embed, k, col)] image so each
   (supertile, embed) loads with a single 128-line DMA: descriptor expansion
   on the Sync engine drops from 3-11 us to ~0.6 us per load, and lines are
   4-17 KB/partition (full line rate);
 - everything rides the Sync/HWDGE queue -- no GPSIMD SWDGE (its library
   preamble delayed kernel start ~5 us and its drain added ~6 us at the end);
 - supertile widths 128/128/256 at the start (fast first matmul, early HAM
   warm-up) and 256/128/128 at the end (short drain chain);
 - the first supertile's x and the embed weights arrive as interleaved
   k-pieces so the first matmul issues ~4 us in;
 - outputs accumulate in one [1, 8192] SBUF tile, stored with a single DMA
   at the end;
 - LN stats: one ACT Square+accumulate per z (PSUM), bn_stats for s1;
   copybacks/relu evicts merged into fewer, larger ACT/DVE ops.
"""

import numpy as np
import ml_dtypes

import concourse.bass as bass
import concourse.mybir as mybir
import concourse.tile as tile
from concourse.bass_utils import run_bass_kernel_spmd

BF16 = ml_dtypes.bfloat16
F32 = np.float32

N_CORES = 8
B = 65536
IN_DIM = 2056
E = 256
EPS = 1e-5
KP = 2176          # feature dim padded to 17*128 (incl. bias row at 2056)
KCH = KP // 128    # 17
ROWS = B // N_CORES   # 8192 rows per core

SUP = [128, 128, 256] + [512] * 14 + [256, 128, 128]   # supertile widths
assert sum(SUP) == ROWS
X2COLS = 2 * KCH * ROWS
K_PIECES0 = [(0, 3), (3, 8), (8, 13), (13, KCH)]   # st0 weight/x interleave

_BF = mybir.dt.bfloat16
_F32 = mybir.dt.float32
_AF = mybir.ActivationFunctionType

# ---- packed bf16 const image column offsets ----
_WM0 = 0
_WC0 = _WM0 + KCH * E          # 4352
_WMID0 = _WC0 + KCH * E        # 8704
_C2W0 = _WMID0 + 4 * E         # 9728
_C3W0 = _C2W0 + 2 * 64         # 9856
_ID0 = _C3W0 + 1               # 9857
_WCOLS = _ID0 + 128            # 9985

# ---- packed f32 const image columns: d broadcast | c2_b | c3_b | eps ----
_FCOLS = E + 3


def _x2_offsets():
    """dram column offset of each (st, embed) segment in the x2 image."""
    offs = {}
    col = 0
    for st, w in enumerate(SUP):
        offs[(st, 0)] = col
        offs[(st, 1)] = col + KCH * w
        col += 2 * KCH * w
    assert col == X2COLS
    return offs


_X2OFF = _x2_offsets()


def _build_nc():
    nc = bass.Bass()

    x2 = nc.dram_tensor("x2", [128, X2COLS], _BF, kind="ExternalInput")
    wpack = nc.dram_tensor("wpack", [128, _WCOLS], _BF, kind="ExternalInput")
    fpack = nc.dram_tensor("fpack", [128, _FCOLS], _F32, kind="ExternalInput")
    out = nc.dram_tensor("out", [1, ROWS], _F32, kind="ExternalOutput")

    from contextlib import ExitStack

    with tile.TileContext(nc) as tc, ExitStack() as ctx:
        consts = ctx.enter_context(tc.tile_pool(name="consts", bufs=1))
        xpool = ctx.enter_context(tc.tile_pool(name="xpool", bufs=3))
        zpool = ctx.enter_context(tc.tile_pool(name="zpool", bufs=3))
        sqpool = ctx.enter_context(tc.tile_pool(name="sqpool", bufs=3))
        hpool = ctx.enter_context(tc.tile_pool(name="hpool", bufs=4))
        spool = ctx.enter_context(tc.tile_pool(name="spool", bufs=6))
        pe_mc = ctx.enter_context(tc.tile_pool(name="pe_mc", bufs=2, space="PSUM"))
        pe_p1 = ctx.enter_context(tc.tile_pool(name="pe_p1", bufs=2, space="PSUM"))
        pe_t = ctx.enter_context(tc.tile_pool(name="pe_t", bufs=2, space="PSUM"))
        pe_small = ctx.enter_context(tc.tile_pool(name="pe_small", bufs=2, space="PSUM"))

        wsb = consts.tile([128, _WCOLS], _BF, tag="wsb")
        fsb = consts.tile([128, _FCOLS], _F32, tag="fsb")
        out_acc = consts.tile([1, ROWS], _F32, tag="out_acc")

        def load_w(c0, c1):
            # weights ride the Scalar-engine HWDGE ring, parallel to the
            # x ring on Sync, so they never delay an x descriptor
            nc.scalar.dma_start(out=wsb[:, c0:c1], in_=wpack[:, c0:c1])

        def load_x_piece(xt, st, e, k0, k1, w):
            src = x2[:, bass.ds(_X2OFF[(st, e)] + k0 * w, (k1 - k0) * w)]
            nc.sync.dma_start(
                out=xt[:, k0:k1, 0:w],
                in_=src.rearrange("p (k c) -> p k c", c=w))

        # views into the packed images
        wm_sb = wsb[:, _WM0:_WC0].rearrange("p (k j) -> p k j", j=E)
        wc_sb = wsb[:, _WC0:_WMID0].rearrange("p (k j) -> p k j", j=E)
        wmid_sb = wsb[:, _WMID0:_C2W0].rearrange("p (k j) -> p k j", j=E)
        c2w_sb = wsb[:, _C2W0:_C3W0].rearrange("p (k j) -> p k j", j=64)
        c3w_sb = wsb[:64, _C3W0:_C3W0 + 1]
        ident = wsb[:, _ID0:_ID0 + 128]
        d_bc = fsb[:, 0:E]
        c2b_ap = fsb[:64, E:E + 1]
        c3b_ap = fsb[:1, E + 1:E + 2]
        eps_ap = fsb[:, E + 2:E + 3]

        def sumsq_psum(ph):
            """sum(x^2) along free axis of a [128, E] PSUM tile (one ACT op;
            DVE cannot dual-read PSUM)."""
            sq = sqpool.tile([128, E], _BF, tag="sq")
            ssq = spool.tile([128, 1], _F32, tag="ssq")
            nc.scalar.activation(out=sq, in_=ph, func=_AF.Square,
                                 accum_out=ssq)
            return ssq

        def rsqrt_mean(ssq, scale):
            """rsqrt(ssq*scale + eps): ACT sqrt + DVE reciprocal."""
            sd = spool.tile([128, 1], _F32, tag="sd")
            nc.scalar.activation(out=sd, in_=ssq, func=_AF.Sqrt,
                                 bias=eps_ap, scale=scale)
            rs = spool.tile([128, 1], _F32, tag="rs")
            nc.vector.reciprocal(out=rs, in_=sd)
            return rs

        h2cats = {}      # st -> [64, 512] bf16 accumulator
        h1tiles = {}     # pair key -> [128, 2, npair, 128] tile

        def stage_a(blk):
            """Embed matmuls for one 128-col block (PE only)."""
            x_m, x_c, bcol = blk["xap"]
            pmc = pe_mc.tile([128, 2, E], _F32, tag="mc")
            for i, (x_sb, w_sb) in enumerate(((x_m, wm_sb), (x_c, wc_sb))):
                for k in range(KCH):
                    nc.tensor.matmul(
                        pmc[:, i, :], lhsT=x_sb[:, k, bcol], rhs=w_sb[:, k, :],
                        start=(k == 0), stop=(k == KCH - 1))
            blk["pmc"] = pmc

        def stage_a2(blk):
            """LN chain + z evict (DVE/ACT only)."""
            pmc = blk.pop("pmc")
            ssq = [sumsq_psum(pmc[:, i, :]) for i in range(2)]
            rs = [rsqrt_mean(s, 1.0 / E) for s in ssq]
            zs = []
            for i in range(2):
                z = zpool.tile([128, E], _BF, tag=f"z{i}")
                nc.scalar.activation(out=z, in_=pmc[:, i, :],
                                     func=_AF.Relu, scale=rs[i])
                zs.append(z)
            blk["z"] = zs

        def stage_b(blk):
            """z transposes on PE + merged copybacks -> hT."""
            pt = pe_t.tile([128, 4, 128], _BF, tag="pt")
            zm, zc = blk.pop("z")
            for chv in range(2):
                nc.tensor.transpose(pt[:, chv, :], zm[:, bass.ts(chv, 128)], ident)
            for chv in range(2):
                nc.tensor.transpose(pt[:, 2 + chv, :], zc[:, bass.ts(chv, 128)], ident)
            ht_m = hpool.tile([128, 2, 128], _BF, tag="hT0")
            ht_c = hpool.tile([128, 2, 128], _BF, tag="hT1")
            nc.scalar.copy(out=ht_m, in_=pt[:, 0:2, :])
            nc.vector.tensor_copy(out=ht_c, in_=pt[:, 2:4, :])
            blk["ht"] = (ht_m, ht_c)

        def stage_c(blk):
            """Mid matmuls + d add + z1 chain."""
            ht_m, ht_c = blk.pop("ht")
            p1 = pe_p1.tile([128, E], _F32, tag="p1")
            nc.tensor.matmul(p1, lhsT=ht_c[:, 0, :], rhs=wmid_sb[:, 0, :], start=True, stop=False)
            nc.tensor.matmul(p1, lhsT=ht_c[:, 1, :], rhs=wmid_sb[:, 1, :], start=False, stop=False)
            nc.tensor.matmul(p1, lhsT=ht_m[:, 0, :], rhs=wmid_sb[:, 2, :], start=False, stop=False)
            nc.tensor.matmul(p1, lhsT=ht_m[:, 1, :], rhs=wmid_sb[:, 3, :], start=False, stop=True)
            s1 = spool.tile([128, E], _F32, tag="s1")
            nc.vector.tensor_add(out=s1, in0=p1, in1=d_bc)
            stats = spool.tile([128, 6], _F32, tag="stats")
            nc.vector.bn_stats(out=stats, in_=s1)
            mv = spool.tile([128, 2], _F32, tag="mv")
            nc.vector.bn_aggr(out=mv, in_=stats)
            rs1 = rsqrt_mean(mv[:, 1:2], 1.0)
            z1 = zpool.tile([128, E], _BF, tag="zmid")
            nc.vector.tensor_scalar_mul(out=z1, in0=s1, scalar1=rs1)
            blk["z1"] = z1

        def stage_d(blk):
            """z1 transposes on PE + h1 = relu(z1T) merged copyback.
            (c1 affine is identity; c1_g is folded into c2_w on host.)"""
            z1 = blk.pop("z1")
            pt = pe_t.tile([128, 4, 128], _BF, tag="pt")
            key = blk["pair"]
            if blk["par"] == 0:
                h1tiles[key] = hpool.tile(
                    [128, 2, blk["npair"], 128], _BF, tag="h1T", name="h1pair")
            h1 = h1tiles[key]
            for chv in range(2):
                nc.tensor.transpose(pt[:, chv, :], z1[:, bass.ts(chv, 128)], ident)
            nc.vector.tensor_scalar_max(
                out=h1[:, :, blk["par"], :], in0=pt[:, 0:2, :], scalar1=0.0)

        def stage_e(blk):
            """c2 matmul per block pair + h2 relu evict."""
            npair = blk["npair"]
            if blk["par"] != npair - 1:
                return
            st = blk["st"]
            h1 = h1tiles.pop(blk["pair"])
            n = npair * 128
            ph2 = pe_small.tile([64, n], _F32, tag="small")
            nc.tensor.matmul(ph2, lhsT=c2w_sb[:, 0, :], rhs=h1[:, 0, :, :], start=True, stop=False)
            nc.tensor.matmul(ph2, lhsT=c2w_sb[:, 1, :], rhs=h1[:, 1, :, :], start=False, stop=True)
            if st not in h2cats:
                h2cats[st] = hpool.tile([64, 512], _BF, tag="h2T", name="h2cat")
            nc.scalar.activation(
                out=h2cats[st][:, bass.ds(blk["pcol"], n)], in_=ph2,
                func=_AF.Relu, bias=c2b_ap)

        def stage_f(blk):
            """Batched c3 matmul over a whole supertile + out_acc write."""
            if not blk["last_in_st"]:
                return
            st, w = blk["st"], blk["stw"]
            po = pe_small.tile([1, w], _F32, tag="small")
            nc.tensor.matmul(po, lhsT=c3w_sb, rhs=h2cats.pop(st)[:, 0:w],
                             start=True, stop=True)
            nc.vector.tensor_scalar_add(
                out=out_acc[:, bass.ds(blk["st0col"], w)], in0=po,
                scalar1=c3b_ap)

        stages = [stage_a2, stage_b, stage_c, stage_d, stage_e, stage_f]
        pipe = []
        col = 0
        # Big-bang start: weights stream on the Scalar HWDGE ring while x
        # streams on Sync.  st1/st2's x is issued BEFORE st0's, so when the
        # first matmul fires (~20 us, after st0 lands) sts 0-2 plus all
        # weights are resident and the PE runs stall-free from its first
        # instruction -- one HAM warm-up, no oscillation (a stuttering early
        # start measured slower than a late clean one).
        load_w(_WM0, _WC0)
        load_w(_WC0, _WMID0)
        load_w(_WMID0, _WCOLS)
        nc.scalar.dma_start(out=fsb, in_=fpack[:])
        xtiles = {}
        for st in (0, 1, 2):
            tm = xpool.tile([128, KCH, 512], _BF, tag="x_m", name=f"xm{st}")
            tc_ = xpool.tile([128, KCH, 512], _BF, tag="x_c", name=f"xc{st}")
            xtiles[st] = (tm, tc_)
        for st in (1, 2, 0):
            for e in range(2):
                load_x_piece(xtiles[st][e], st, e, 0, KCH, SUP[st])
        for st, w in enumerate(SUP):
            nb = w // 128
            if st in xtiles:
                x_m, x_c = xtiles.pop(st)
            else:
                x_m = xpool.tile([128, KCH, 512], _BF, tag="x_m")
                x_c = xpool.tile([128, KCH, 512], _BF, tag="x_c")
                # two k-halves per embed: finer arrival granularity (any
                # residual DMA-deficit stall stays under the HAM MID window)
                for xt, e in ((x_m, 0), (x_c, 1)):
                    load_x_piece(xt, st, e, 0, 9, w)
                    load_x_piece(xt, st, e, 9, KCH, w)

            for bb in range(nb):
                blk = {
                    "st": st, "stw": w, "st0col": col,
                    "pair": (st, bb // 2), "par": bb % 2,
                    "npair": min(2, nb - (bb // 2) * 2),
                    "pcol": (bb // 2) * 256,
                    "last_in_st": bb == nb - 1,
                    "xap": (x_m, x_c, bass.ts(bb, 128)),
                }
                stage_a(blk)
                pipe.append(blk)
                for depth, fn in enumerate(stages, start=2):
                    if len(pipe) >= depth:
                        fn(pipe[-depth])
                if len(pipe) > len(stages) + 1:
                    pipe.pop(0)
            col += w
        # drain: stage j (depth j+2) still owes the last j+1 blocks
        for j, fn in enumerate(stages):
            for blk in pipe[-(j + 1):]:
                fn(blk)
        nc.scalar.dma_start(out=out[:, :], in_=out_acc)

    return nc


def _legalize_waits(nc):
    """Split multi-semaphore waits: this walrus build accepts at most one
    sync-wait per instruction (two on EventSemaphore), so excess waits are
    hoisted into preceding EventSemaphore instructions on the same engine."""
    for bb in nc.main_func.blocks:
        new_insts = []
        changed = False
        for inst in bb.instructions:
            si = inst.sync_info
            if si is not None and si.on_wait:
                cap = 2 if isinstance(inst, mybir.InstEventSemaphore) else 1
                waits = list(si.on_wait)
                while len(waits) > cap:
                    spill, waits = waits[:2], waits[2:]
                    ev = mybir.InstEventSemaphore(
                        name=nc.get_next_instruction_name(),
                        ins=[], outs=[],
                        engine=inst.engine,
                        sync_info=mybir.SyncInfo(on_wait=spill, on_update=[]),
                    )
                    new_insts.append(ev)
                    changed = True
                si.on_wait = waits
            new_insts.append(inst)
        if changed:
            bb.instructions[:] = new_insts


_NC_CACHE = {}


def _get_nc():
    if "nc" not in _NC_CACHE:
        nc = _build_nc()
        _legalize_waits(nc)
        _NC_CACHE["nc"] = nc
    return _NC_CACHE["nc"]


def _fold_weights(inp):
    f8 = lambda k: np.asarray(inp[k]).astype(np.float64)
    P_c = np.eye(E) - 1.0 / E

    me_w, me_b = f8("me_w"), f8("me_b")
    ce_w, ce_b = f8("ce_w"), f8("ce_b")
    Wm = np.zeros((KP, E))
    Wm[:IN_DIM] = me_w @ P_c
    Wm[IN_DIM] = me_b @ P_c
    Wc = np.zeros((KP, E))
    Wc[:IN_DIM] = ce_w @ P_c
    Wc[IN_DIM] = ce_b @ P_c

    c1_w, c1_b = f8("c1_w"), f8("c1_b")
    A0 = f8("ca_in_w")[:, 2 * E:] @ f8("ca_out_w")
    a0 = f8("ca_in_b")[2 * E:] @ f8("ca_out_w") + f8("ca_out_b")
    S0 = f8("sa_in_w")[:, 2 * E:] @ f8("sa_out_w")
    s0 = f8("sa_in_b")[2 * E:] @ f8("sa_out_w") + f8("sa_out_b")
    CA = (A0 @ c1_w[:E]) @ P_c
    CM = (S0 @ c1_w[E:]) @ P_c
    d = (a0 @ c1_w[:E] + s0 @ c1_w[E:] + c1_b) @ P_c

    # fold c1's affine (g, be) into the c2 projection: with be == 0 and
    # g > 0, relu(g*z + be) @ c2_w == relu(z) @ (g[:,None] * c2_w)
    c1_g, c1_be = f8("c1_g"), f8("c1_be")
    assert np.all(c1_be == 0.0) and np.all(c1_g > 0.0)
    c2_w = c1_g[:, None] * f8("c2_w")

    # ---- bf16 packed image ----
    w = np.zeros((128, _WCOLS), BF16)

    def chunked(mat, ncols):       # [k*128, ncols] -> [128, k*ncols]
        k = mat.shape[0] // 128
        return mat.reshape(k, 128, ncols).transpose(1, 0, 2).reshape(128, k * ncols)

    w[:, _WM0:_WC0] = chunked(Wm, E).astype(BF16)
    w[:, _WC0:_WMID0] = chunked(Wc, E).astype(BF16)
    w[:, _WMID0:_C2W0] = chunked(np.vstack([CA, CM]), E).astype(BF16)
    w[:, _C2W0:_C3W0] = chunked(c2_w, 64).astype(BF16)
    w[:64, _C3W0:_C3W0 + 1] = f8("c3_w").astype(BF16)
    w[:, _ID0:_ID0 + 128] = np.eye(128, dtype=BF16)

    # ---- f32 packed image: d broadcast | c2_b | c3_b | eps ----
    f = np.zeros((128, _FCOLS), F32)
    f[:, 0:E] = d.astype(F32)[None, :]
    f[:64, E] = np.asarray(inp["c2_b"]).astype(F32)
    f[0, E + 1] = float(np.asarray(inp["c3_b"]).reshape(-1)[0])
    f[:, E + 2] = EPS
    return {"wpack": w, "fpack": f}


def _shard_x(x):
    """x [B, 2, IN_DIM] f32 -> per-core x2 image [128, X2COLS] bf16.

    Layout: for each supertile st (width w) and embed e, a contiguous
    [128, KCH*w] segment holding that supertile's feature-major x, k-major
    within the segment -- so every (st, e) DMA is one 4-17 KB line per
    partition, and k-range sub-slices stay contiguous."""
    maps = []
    for i in range(N_CORES):
        sl = x[i * ROWS:(i + 1) * ROWS]          # [ROWS, 2, IN_DIM]
        xe = []
        for e in range(2):
            t = np.zeros((KP, ROWS), BF16)
            t[:IN_DIM] = np.ascontiguousarray(sl[:, e, :]).astype(BF16).T
            t[IN_DIM] = 1
            xe.append(t)
        segs = []
        col = 0
        for st, w in enumerate(SUP):
            for e in range(2):
                # [KCH, 128, w] -> [128, KCH, w] -> [128, KCH*w]
                seg = xe[e][:, col:col + w].reshape(KCH, 128, w)
                segs.append(seg.transpose(1, 0, 2).reshape(128, KCH * w))
            col += w
        x2 = np.concatenate(segs, axis=1)
        assert x2.shape == (128, X2COLS)
        maps.append(np.ascontiguousarray(x2))
    return maps


def kernel(**inputs):
    x = np.asarray(inputs["x"], dtype=np.float32)
    weights = _fold_weights(inputs)
    shards = _shard_x(x)
    in_maps = [{"x2": x2, **weights} for x2 in shards]

    nc = _get_nc()
    res = run_bass_kernel_spmd(nc, in_maps, list(range(N_CORES)))
    outs = [np.asarray(r["out"]).reshape(ROWS) for r in res.results]
    return np.concatenate(outs).reshape(B, 1).astype(np.float32)


# revision 26
# speedup vs baseline: 1.0275x; 1.0235x over previous
"""Trainium2 Bass kernel for nn_CrossAttentionClassifier.

Strategy
--------
The reference network with q_len = kv_len = 1 attention degenerates into a
pure MLP:

    z_m = mut @ Wm' + bm'          (centered: LN mean-subtraction folded into W)
    z_c = ctx @ Wc' + bc'
    h_m = relu(z_m * rsqrt(mean(z_m^2)+eps)) ; h_c likewise
    pre1 = h_c @ CA + h_m @ CM + d (attention V/out projections + c1 folded)
    z1   = pre1 * rsqrt(mean(pre1^2)+eps)
    h1   = relu(z1)                (c1_g==1, c1_be==0; g folded into c2_w)
    h2   = relu(h1 @ c2_w + c2_b)
    out  = h2 @ c3_w + c3_b

All weight folding (products of the tiny 256x256 projection chains and the
centering projector I - 11^T/256) happens on host in float64; the batch-heavy
work runs on 8 NeuronCores, data-parallel over the 65536-row batch.

Device layout: batch on SBUF partitions, features on the free axis, so both
layernorms reduce along the free dimension.  Activations are transposed
128x128 via the PE between layers so the next matmul's contraction dim lands
on partitions.  All matmuls run in bf16 (fp32 PSUM accumulate).

Scheduling (trace-driven; the PE matmul stream is the bottleneck at ~107 ns
per N=256 MM, i.e. the 1 col/cycle floor):
 - supertile 0's x arrives from a block-major packed copy (x0pack) with
   4.3 KB/partition DMA lines, so the embed matmuls start as soon as the
   first block + embed weights land and then run without long stalls (one
   HAM clock-gate warm-up, no oscillation);
 - weight/const loads and output stores ride the GPSIMD DMA queue so they
   never stall the in-order x-input ring on the sync queue;
 - x tiles are triple-buffered and loaded in two k-halves;
 - the last supertiles are 256/128/128 columns so the end-of-kernel drain
   chain is short;
 - LN stats: one ACT Square+accumulate per z (PSUM), bn_stats for s1;
   copybacks/relu evicts merged into fewer, larger ACT/DVE ops.
"""

import numpy as np
import ml_dtypes

import concourse.bass as bass
import concourse.mybir as mybir
import concourse.tile as tile
from concourse.bass_utils import run_bass_kernel_spmd

BF16 = ml_dtypes.bfloat16
F32 = np.float32

N_CORES = 8
B = 65536
IN_DIM = 2056
E = 256
EPS = 1e-5
KP = 2176          # feature dim padded to 17*128 (incl. bias row at 2056)
KCH = KP // 128    # 17
ROWS = B // N_CORES   # 8192 rows per core

SUP = [512] * 15 + [256, 128, 128]   # supertile widths
assert sum(SUP) == ROWS
NBLK0 = SUP[0] // 128                # blocks in supertile 0 (x0pack-fed)
KHALF = 9                            # k-chunk split point for x half-DMAs

_BF = mybir.dt.bfloat16
_F32 = mybir.dt.float32
_AF = mybir.ActivationFunctionType

# ---- packed bf16 const image column offsets ----
_WM0 = 0
_WC0 = _WM0 + KCH * E          # 4352
_WMID0 = _WC0 + KCH * E        # 8704
_C2W0 = _WMID0 + 4 * E         # 9728
_C3W0 = _C2W0 + 2 * 64         # 9856
_ID0 = _C3W0 + 1               # 9857
_WCOLS = _ID0 + 128            # 9985

# ---- packed f32 const image columns: d broadcast | c2_b | c3_b | eps ----
_FCOLS = E + 3


def _build_nc():
    nc = bass.Bass()

    xm = nc.dram_tensor("xm", [KP, ROWS], _BF, kind="ExternalInput")
    xc = nc.dram_tensor("xc", [KP, ROWS], _BF, kind="ExternalInput")
    x0 = nc.dram_tensor("x0", [128, NBLK0 * 2 * KCH * 128], _BF,
                        kind="ExternalInput")
    wpack = nc.dram_tensor("wpack", [128, _WCOLS], _BF, kind="ExternalInput")
    fpack = nc.dram_tensor("fpack", [128, _FCOLS], _F32, kind="ExternalInput")
    out = nc.dram_tensor("out", [1, ROWS], _F32, kind="ExternalOutput")

    from contextlib import ExitStack

    with tile.TileContext(nc) as tc, ExitStack() as ctx:
        consts = ctx.enter_context(tc.tile_pool(name="consts", bufs=1))
        x0pool = ctx.enter_context(tc.tile_pool(name="x0pool", bufs=1))
        xpool = ctx.enter_context(tc.tile_pool(name="xpool", bufs=3))
        zpool = ctx.enter_context(tc.tile_pool(name="zpool", bufs=3))
        sqpool = ctx.enter_context(tc.tile_pool(name="sqpool", bufs=3))
        hpool = ctx.enter_context(tc.tile_pool(name="hpool", bufs=4))
        spool = ctx.enter_context(tc.tile_pool(name="spool", bufs=6))
        opool = ctx.enter_context(tc.tile_pool(name="opool", bufs=3))
        pe_mc = ctx.enter_context(tc.tile_pool(name="pe_mc", bufs=2, space="PSUM"))
        pe_p1 = ctx.enter_context(tc.tile_pool(name="pe_p1", bufs=2, space="PSUM"))
        pe_t = ctx.enter_context(tc.tile_pool(name="pe_t", bufs=2, space="PSUM"))
        pe_small = ctx.enter_context(tc.tile_pool(name="pe_small", bufs=2, space="PSUM"))

        wsb = consts.tile([128, _WCOLS], _BF, tag="wsb")
        fsb = consts.tile([128, _FCOLS], _F32, tag="fsb")

        # weight/const DMAs on the GPSIMD queue (parallel to the x ring)
        nc.gpsimd.dma_start(out=wsb[:, _WM0:_WC0], in_=wpack[:, _WM0:_WC0])
        nc.gpsimd.dma_start(out=wsb[:, _WC0:_WMID0], in_=wpack[:, _WC0:_WMID0])
        nc.gpsimd.dma_start(out=wsb[:, _WMID0:], in_=wpack[:, _WMID0:])
        nc.gpsimd.dma_start(out=fsb, in_=fpack[:])

        # views into the packed images
        wm_sb = wsb[:, _WM0:_WC0].rearrange("p (k j) -> p k j", j=E)
        wc_sb = wsb[:, _WC0:_WMID0].rearrange("p (k j) -> p k j", j=E)
        wmid_sb = wsb[:, _WMID0:_C2W0].rearrange("p (k j) -> p k j", j=E)
        c2w_sb = wsb[:, _C2W0:_C3W0].rearrange("p (k j) -> p k j", j=64)
        c3w_sb = wsb[:64, _C3W0:_C3W0 + 1]
        ident = wsb[:, _ID0:_ID0 + 128]
        d_bc = fsb[:, 0:E]
        c2b_ap = fsb[:64, E:E + 1]
        c3b_ap = fsb[:1, E + 1:E + 2]
        eps_ap = fsb[:, E + 2:E + 3]

        def sumsq_psum(ph):
            """sum(x^2) along free axis of a [128, E] PSUM tile (one ACT op;
            DVE cannot dual-read PSUM)."""
            sq = sqpool.tile([128, E], _BF, tag="sq")
            ssq = spool.tile([128, 1], _F32, tag="ssq")
            nc.scalar.activation(out=sq, in_=ph, func=_AF.Square,
                                 accum_out=ssq)
            return ssq

        def rsqrt_mean(ssq, scale):
            """rsqrt(ssq*scale + eps): ACT sqrt + DVE reciprocal."""
            sd = spool.tile([128, 1], _F32, tag="sd")
            nc.scalar.activation(out=sd, in_=ssq, func=_AF.Sqrt,
                                 bias=eps_ap, scale=scale)
            rs = spool.tile([128, 1], _F32, tag="rs")
            nc.vector.reciprocal(out=rs, in_=sd)
            return rs

        out_tiles = {}   # st -> [1, 512] out tile
        h2cats = {}      # st -> [64, 512] bf16 accumulator
        h1tiles = {}     # pair key -> [128, 2, npair, 128] tile

        def stage_a(blk):
            """Embed matmuls for one 128-col block (PE only)."""
            x_m, x_c, bcol = blk["xap"]
            pmc = pe_mc.tile([128, 2, E], _F32, tag="mc")
            for i, (x_sb, w_sb) in enumerate(((x_m, wm_sb), (x_c, wc_sb))):
                for k in range(KCH):
                    nc.tensor.matmul(
                        pmc[:, i, :], lhsT=x_sb[:, k, bcol], rhs=w_sb[:, k, :],
                        start=(k == 0), stop=(k == KCH - 1))
            blk["pmc"] = pmc

        def stage_a2(blk):
            """LN chain + z evict (DVE/ACT only)."""
            pmc = blk.pop("pmc")
            ssq = [sumsq_psum(pmc[:, i, :]) for i in range(2)]
            rs = [rsqrt_mean(s, 1.0 / E) for s in ssq]
            zs = []
            for i in range(2):
                z = zpool.tile([128, E], _BF, tag=f"z{i}")
                nc.scalar.activation(out=z, in_=pmc[:, i, :],
                                     func=_AF.Relu, scale=rs[i])
                zs.append(z)
            blk["z"] = zs

        def stage_b(blk):
            """z transposes on PE + merged copybacks -> hT."""
            pt = pe_t.tile([128, 4, 128], _BF, tag="pt")
            zm, zc = blk.pop("z")
            for chv in range(2):
                nc.tensor.transpose(pt[:, chv, :], zm[:, bass.ts(chv, 128)], ident)
            for chv in range(2):
                nc.tensor.transpose(pt[:, 2 + chv, :], zc[:, bass.ts(chv, 128)], ident)
            ht_m = hpool.tile([128, 2, 128], _BF, tag="hT0")
            ht_c = hpool.tile([128, 2, 128], _BF, tag="hT1")
            nc.scalar.copy(out=ht_m, in_=pt[:, 0:2, :])
            nc.vector.tensor_copy(out=ht_c, in_=pt[:, 2:4, :])
            blk["ht"] = (ht_m, ht_c)

        def stage_c(blk):
            """Mid matmuls + d add + z1 chain."""
            ht_m, ht_c = blk.pop("ht")
            p1 = pe_p1.tile([128, E], _F32, tag="p1")
            nc.tensor.matmul(p1, lhsT=ht_c[:, 0, :], rhs=wmid_sb[:, 0, :], start=True, stop=False)
            nc.tensor.matmul(p1, lhsT=ht_c[:, 1, :], rhs=wmid_sb[:, 1, :], start=False, stop=False)
            nc.tensor.matmul(p1, lhsT=ht_m[:, 0, :], rhs=wmid_sb[:, 2, :], start=False, stop=False)
            nc.tensor.matmul(p1, lhsT=ht_m[:, 1, :], rhs=wmid_sb[:, 3, :], start=False, stop=True)
            s1 = spool.tile([128, E], _F32, tag="s1")
            nc.vector.tensor_add(out=s1, in0=p1, in1=d_bc)
            stats = spool.tile([128, 6], _F32, tag="stats")
            nc.vector.bn_stats(out=stats, in_=s1)
            mv = spool.tile([128, 2], _F32, tag="mv")
            nc.vector.bn_aggr(out=mv, in_=stats)
            rs1 = rsqrt_mean(mv[:, 1:2], 1.0)
            z1 = zpool.tile([128, E], _BF, tag="zmid")
            nc.vector.tensor_scalar_mul(out=z1, in0=s1, scalar1=rs1)
            blk["z1"] = z1

        def stage_d(blk):
            """z1 transposes on PE + h1 = relu(z1T) merged copyback.
            (c1 affine is identity; c1_g is folded into c2_w on host.)"""
            z1 = blk.pop("z1")
            pt = pe_t.tile([128, 4, 128], _BF, tag="pt")
            key = blk["pair"]
            if blk["par"] == 0:
                h1tiles[key] = hpool.tile(
                    [128, 2, blk["npair"], 128], _BF, tag="h1T", name="h1pair")
            h1 = h1tiles[key]
            for chv in range(2):
                nc.tensor.transpose(pt[:, chv, :], z1[:, bass.ts(chv, 128)], ident)
            nc.vector.tensor_scalar_max(
                out=h1[:, :, blk["par"], :], in0=pt[:, 0:2, :], scalar1=0.0)

        def stage_e(blk):
            """c2 matmul per block pair + h2 relu evict."""
            npair = blk["npair"]
            if blk["par"] != npair - 1:
                return
            st = blk["st"]
            h1 = h1tiles.pop(blk["pair"])
            n = npair * 128
            ph2 = pe_small.tile([64, n], _F32, tag="small")
            nc.tensor.matmul(ph2, lhsT=c2w_sb[:, 0, :], rhs=h1[:, 0, :, :], start=True, stop=False)
            nc.tensor.matmul(ph2, lhsT=c2w_sb[:, 1, :], rhs=h1[:, 1, :, :], start=False, stop=True)
            if st not in h2cats:
                h2cats[st] = hpool.tile([64, 512], _BF, tag="h2T", name="h2cat")
            nc.scalar.activation(
                out=h2cats[st][:, bass.ds(blk["pcol"], n)], in_=ph2,
                func=_AF.Relu, bias=c2b_ap)

        def stage_f(blk):
            """Batched c3 matmul over a whole supertile + output write."""
            if not blk["last_in_st"]:
                return
            st, w = blk["st"], blk["stw"]
            po = pe_small.tile([1, w], _F32, tag="small")
            nc.tensor.matmul(po, lhsT=c3w_sb, rhs=h2cats.pop(st)[:, 0:w],
                             start=True, stop=True)
            ot = out_tiles.pop(st)
            nc.vector.tensor_scalar_add(out=ot[:, 0:w], in0=po, scalar1=c3b_ap)
            nc.gpsimd.dma_start(out=out[:, bass.ds(blk["st0col"], w)],
                                in_=ot[:, 0:w])

        stages = [stage_a2, stage_b, stage_c, stage_d, stage_e, stage_f]
        pipe = []
        col = 0
        for st, w in enumerate(SUP):
            nb = w // 128
            if st == 0:
                x0m, x0c = [], []
                for b in range(NBLK0):
                    tm = x0pool.tile([128, KCH, 128], _BF, tag=f"x0m{b}")
                    tc_ = x0pool.tile([128, KCH, 128], _BF, tag=f"x0c{b}")
                    for e, t in ((0, tm), (1, tc_)):
                        nc.sync.dma_start(
                            out=t,
                            in_=x0[:, bass.ds((b * 2 + e) * KCH * 128,
                                              KCH * 128)].rearrange(
                                "p (k c) -> p k c", c=128))
                    x0m.append(tm)
                    x0c.append(tc_)
            else:
                x_m = xpool.tile([128, KCH, 512], _BF, tag="x_m")
                x_c = xpool.tile([128, KCH, 512], _BF, tag="x_c")
                cols = bass.ds(col, w)
                for xt, xd in ((x_m, xm), (x_c, xc)):
                    for k0, k1 in ((0, KHALF), (KHALF, KCH)):
                        nc.sync.dma_start(
                            out=xt[:, k0:k1, 0:w],
                            in_=xd[k0 * 128:k1 * 128, cols].rearrange(
                                "(k p) c -> p k c", p=128))
            out_tiles[st] = opool.tile([1, 512], _F32, tag="out_sb",
                                       name="out_sb")

            for bb in range(nb):
                blk = {
                    "st": st, "stw": w, "st0col": col,
                    "pair": (st, bb // 2), "par": bb % 2,
                    "npair": min(2, nb - (bb // 2) * 2),
                    "pcol": (bb // 2) * 256,
                    "last_in_st": bb == nb - 1,
                }
                if st == 0:
                    blk["xap"] = (x0m[bb], x0c[bb], bass.ds(0, 128))
                else:
                    blk["xap"] = (x_m, x_c, bass.ts(bb, 128))
                stage_a(blk)
                pipe.append(blk)
                for depth, fn in enumerate(stages, start=2):
                    if len(pipe) >= depth:
                        fn(pipe[-depth])
                if len(pipe) > len(stages) + 1:
                    pipe.pop(0)
            col += w
        # drain: stage j (depth j+2) still owes the last j+1 blocks
        for j, fn in enumerate(stages):
            for blk in pipe[-(j + 1):]:
                fn(blk)

    return nc


def _legalize_waits(nc):
    """Split multi-semaphore waits: this walrus build accepts at most one
    sync-wait per instruction (two on EventSemaphore), so excess waits are
    hoisted into preceding EventSemaphore instructions on the same engine."""
    for bb in nc.main_func.blocks:
        new_insts = []
        changed = False
        for inst in bb.instructions:
            si = inst.sync_info
            if si is not None and si.on_wait:
                cap = 2 if isinstance(inst, mybir.InstEventSemaphore) else 1
                waits = list(si.on_wait)
                while len(waits) > cap:
                    spill, waits = waits[:2], waits[2:]
                    ev = mybir.InstEventSemaphore(
                        name=nc.get_next_instruction_name(),
                        ins=[], outs=[],
                        engine=inst.engine,
                        sync_info=mybir.SyncInfo(on_wait=spill, on_update=[]),
                    )
                    new_insts.append(ev)
                    changed = True
                si.on_wait = waits
            new_insts.append(inst)
        if changed:
            bb.instructions[:] = new_insts


_NC_CACHE = {}


def _get_nc():
    if "nc" not in _NC_CACHE:
        nc = _build_nc()
        _legalize_waits(nc)
        _NC_CACHE["nc"] = nc
    return _NC_CACHE["nc"]


def _fold_weights(inp):
    f8 = lambda k: np.asarray(inp[k]).astype(np.float64)
    P_c = np.eye(E) - 1.0 / E

    me_w, me_b = f8("me_w"), f8("me_b")
    ce_w, ce_b = f8("ce_w"), f8("ce_b")
    Wm = np.zeros((KP, E))
    Wm[:IN_DIM] = me_w @ P_c
    Wm[IN_DIM] = me_b @ P_c
    Wc = np.zeros((KP, E))
    Wc[:IN_DIM] = ce_w @ P_c
    Wc[IN_DIM] = ce_b @ P_c

    c1_w, c1_b = f8("c1_w"), f8("c1_b")
    A0 = f8("ca_in_w")[:, 2 * E:] @ f8("ca_out_w")
    a0 = f8("ca_in_b")[2 * E:] @ f8("ca_out_w") + f8("ca_out_b")
    S0 = f8("sa_in_w")[:, 2 * E:] @ f8("sa_out_w")
    s0 = f8("sa_in_b")[2 * E:] @ f8("sa_out_w") + f8("sa_out_b")
    CA = (A0 @ c1_w[:E]) @ P_c
    CM = (S0 @ c1_w[E:]) @ P_c
    d = (a0 @ c1_w[:E] + s0 @ c1_w[E:] + c1_b) @ P_c

    # fold c1's affine (g, be) into the c2 projection: with be == 0 and
    # g > 0, relu(g*z + be) @ c2_w == relu(z) @ (g[:,None] * c2_w)
    c1_g, c1_be = f8("c1_g"), f8("c1_be")
    assert np.all(c1_be == 0.0) and np.all(c1_g > 0.0)
    c2_w = c1_g[:, None] * f8("c2_w")

    # ---- bf16 packed image ----
    w = np.zeros((128, _WCOLS), BF16)

    def chunked(mat, ncols):       # [k*128, ncols] -> [128, k*ncols]
        k = mat.shape[0] // 128
        return mat.reshape(k, 128, ncols).transpose(1, 0, 2).reshape(128, k * ncols)

    w[:, _WM0:_WC0] = chunked(Wm, E).astype(BF16)
    w[:, _WC0:_WMID0] = chunked(Wc, E).astype(BF16)
    w[:, _WMID0:_C2W0] = chunked(np.vstack([CA, CM]), E).astype(BF16)
    w[:, _C2W0:_C3W0] = chunked(c2_w, 64).astype(BF16)
    w[:64, _C3W0:_C3W0 + 1] = f8("c3_w").astype(BF16)
    w[:, _ID0:_ID0 + 128] = np.eye(128, dtype=BF16)

    # ---- f32 packed image: d broadcast | c2_b | c3_b | eps ----
    f = np.zeros((128, _FCOLS), F32)
    f[:, 0:E] = d.astype(F32)[None, :]
    f[:64, E] = np.asarray(inp["c2_b"]).astype(F32)
    f[0, E + 1] = float(np.asarray(inp["c3_b"]).reshape(-1)[0])
    f[:, E + 2] = EPS
    return {"wpack": w, "fpack": f}


def _shard_x(x):
    """x [B, 2, IN_DIM] f32 -> per-core (xm, xc, x0pack) in bf16.

    xm/xc are feature-major [KP, ROWS]; x0pack is a block-major copy of the
    first SUP[0] columns laid out [128, (b, e, k, c)] so each (block, embed)
    loads with one fully-contiguous-line DMA."""
    maps = []
    for i in range(N_CORES):
        sl = x[i * ROWS:(i + 1) * ROWS]          # [ROWS, 2, IN_DIM]
        xm = np.zeros((KP, ROWS), BF16)
        xc = np.zeros((KP, ROWS), BF16)
        xm[:IN_DIM] = np.ascontiguousarray(sl[:, 0, :]).astype(BF16).T
        xm[IN_DIM] = 1
        xc[:IN_DIM] = np.ascontiguousarray(sl[:, 1, :]).astype(BF16).T
        xc[IN_DIM] = 1
        w0 = SUP[0]
        # [k, p, b, c] -> [p, b, k, c]
        bm = xm[:, :w0].reshape(KCH, 128, NBLK0, 128).transpose(1, 2, 0, 3)
        bc = xc[:, :w0].reshape(KCH, 128, NBLK0, 128).transpose(1, 2, 0, 3)
        x0 = np.stack([bm, bc], axis=2)          # [p, b, e, k, c]
        x0 = np.ascontiguousarray(x0).reshape(128, NBLK0 * 2 * KCH * 128)
        maps.append((xm, xc, x0))
    return maps


def kernel(**inputs):
    x = np.asarray(inputs["x"], dtype=np.float32)
    weights = _fold_weights(inputs)
    shards = _shard_x(x)
    in_maps = [{"xm": xm, "xc": xc, "x0": x0, **weights}
               for xm, xc, x0 in shards]

    nc = _get_nc()
    res = run_bass_kernel_spmd(nc, in_maps, list(range(N_CORES)))
    outs = [np.asarray(r["out"]).reshape(ROWS) for r in res.results]
    return np.concatenate(outs).reshape(B, 1).astype(np.float32)
